# revision 18
# baseline (speedup 1.0000x reference)
"""Trainium2 Bass kernel for a cross-attention block with 3D-coordinate RoPE.

Module: q/k/v projections of x [B,Tq,D] against memory [B,Tk,D], 3D-coord
rotary embedding on q/k, softmax(q k^T / sqrt(Hd)) v, output projection.
B=2, Tq=1024, Tk=2048, D=1536, 16 heads x 96.

Sharding: 8 cores = (2 batches) x (4 head-groups of 4 heads). Each core
computes its heads end-to-end plus a partial output projection; the host
sums the 4 partials per batch. Biases bv/bo are folded in on the host
(attention rows sum to one), bq/bk are added on-device during PSUM
eviction.

Layout: feature-major ("transposed") on device. Scores are computed
transposed (S^T = k q^T) so the PV matmul needs no on-chip transposes;
softmax denominators come from a ones-column appended to v; the
per-query normalization is broadcast across partitions with a tiny
K=4 one-hot matmul.
"""

import os
import sys

sys.path.insert(0, "/opt/trn_rl_repo")

import numpy as np
from contextlib import ExitStack

import concourse.bass as bass
import concourse.tile as tile
from concourse import bacc, mybir
from concourse.bass_utils import run_bass_kernel_spmd

# ---------------------------------------------------------------- constants
B = 2
TQ = 1024
TK = 2048
D = 1536
NH = 16
HD = 96
ROPE_HALF = HD // 2           # 48
FREQ_PER_AXIS = ROPE_HALF // 3  # 16
ROPE_BASE = 10000.0
NH_CORE = 4                   # heads per core
HG = NH_CORE * HD             # 384 features per core
KC = D // 128                 # 12 contraction chunks
MTILES = D // 128             # 12 output-row tiles of the o-projection
SCALE = 1.0 / float(np.sqrt(HD))
N_CORES = 8
VW = HD + 1                   # 97: head-dim + ones column

F32 = mybir.dt.float32

_MM_DT_NAME = os.environ.get("KMM_DTYPE", "f32r")
MM_DT = {"f32r": mybir.dt.float32r, "f32": mybir.dt.float32}[_MM_DT_NAME]


def _mm(ap):
    """View an fp32 AP with the matmul compute dtype (bitcast, no rounding)."""
    if MM_DT is F32:
        return ap
    return ap.bitcast(MM_DT)


# ---------------------------------------------------------------- bass build
def _build_nc():
    nc = bacc.Bacc(trn_type="TRN2", target_bir_lowering=False, debug=False)

    io = {}
    def dram_in(name, shape):
        io[name] = nc.dram_tensor(name, list(shape), F32, kind="ExternalInput").ap()
    dram_in("xT", [D, TQ])
    dram_in("memT", [D, TK])
    dram_in("wqT", [D, HG])       # columns of Wq^T for this head group
    dram_in("wkT", [D, HG])
    dram_in("wvT", [D, HG])
    dram_in("woT", [HG, D])       # rows of Wo^T for this head group
    dram_in("bq4", [HD, NH_CORE])
    dram_in("bk4", [HD, NH_CORE])
    dram_in("cqE", [HD, TQ])      # cos table, feature-major, q side
    dram_in("sqE", [HD, TQ])      # sign-folded sin table, q side
    dram_in("ckE", [HD, TK])
    dram_in("skE", [HD, TK])
    dram_in("perm", [HD, HD])     # lhsT of the half-rotation swap
    dram_in("ones1", [1, 128])
    dram_in("ones4", [128, NH_CORE])
    oT = nc.dram_tensor("oT", [D, TQ], F32, kind="ExternalOutput").ap()

    with tile.TileContext(nc) as tc, ExitStack() as ctx:
        _body(ctx, tc, io, oT)
    nc.compile()
    return nc


def _body(ctx, tc, io, oT):
    nc = tc.nc
    P = 128
    Exp = mybir.ActivationFunctionType.Exp
    Ident = mybir.ActivationFunctionType.Identity

    const = ctx.enter_context(tc.tile_pool(name="const", bufs=1))
    resident = ctx.enter_context(tc.tile_pool(name="resident", bufs=1))

    # ---- constants -------------------------------------------------------
    perm_t = const.tile([HD, HD], MM_DT, name="perm_t")
    nc.sync.dma_start(out=perm_t[:], in_=_mm(io["perm"][:]))
    ones1_t = const.tile([1, P], MM_DT, name="ones1_t")
    nc.sync.dma_start(out=ones1_t[:], in_=_mm(io["ones1"][:]))
    bq_t = const.tile([HD, NH_CORE], F32, name="bq_t")
    nc.sync.dma_start(out=bq_t[:], in_=io["bq4"][:])
    bk_t = const.tile([HD, NH_CORE], F32, name="bk_t")
    nc.sync.dma_start(out=bk_t[:], in_=io["bk4"][:])
    cq_t = const.tile([HD, TQ], F32, name="cq_t")
    nc.sync.dma_start(out=cq_t[:], in_=io["cqE"][:])
    sq_t = const.tile([HD, TQ], F32, name="sq_t")
    nc.sync.dma_start(out=sq_t[:], in_=io["sqE"][:])
    ck_t = const.tile([HD, TK], F32, name="ck_t")
    nc.sync.dma_start(out=ck_t[:], in_=io["ckE"][:])
    sk_t = const.tile([HD, TK], F32, name="sk_t")
    nc.sync.dma_start(out=sk_t[:], in_=io["skE"][:])

    # ---- long-lived results ---------------------------------------------
    qT = [resident.tile([HD, TQ], MM_DT, name=f"qT{h}", tag=f"qT{h}")
          for h in range(NH_CORE)]
    kT = [resident.tile([HD, TK], MM_DT, name=f"kT{h}", tag=f"kT{h}")
          for h in range(NH_CORE)]
    vst = [resident.tile([P, NH_CORE * VW], MM_DT, name=f"vst{m}", tag=f"vst{m}")
           for m in range(TK // P)]
    # ones columns of v (one strided DMA per m-tile)
    for m in range(TK // P):
        ones_cols = vst[m].rearrange("p (h c) -> p h c", c=VW)[:, :, HD:HD + 1]
        nc.sync.dma_start(out=ones_cols,
                          in_=_mm(io["ones4"][:]).rearrange("p (h c) -> p h c", c=1))

    # ---- phase Q: q^T = Wq_h @ x^T (c-outer, 8 psum banks) ---------------
    with ExitStack() as qctx:
        psq_pool = qctx.enter_context(
            tc.tile_pool(name="psq", bufs=NH_CORE, space="PSUM"))
        xq_pool = qctx.enter_context(tc.tile_pool(name="xq", bufs=3))
        wq_pool = qctx.enter_context(tc.tile_pool(name="wq", bufs=3))
        psq = [psq_pool.tile([HD, TQ], F32, name=f"psq{h}", tag="psq")
               for h in range(NH_CORE)]
        for c in range(KC):
            xc = xq_pool.tile([P, TQ], MM_DT, name="xc", tag="xc")
            nc.sync.dma_start(out=xc[:], in_=_mm(io["xT"][c * P:(c + 1) * P, :]))
            wqc = wq_pool.tile([P, HG], MM_DT, name="wqc", tag="wqc")
            nc.sync.dma_start(out=wqc[:], in_=_mm(io["wqT"][c * P:(c + 1) * P, :]))
            for h in range(NH_CORE):
                lhs = wqc[:, h * HD:(h + 1) * HD]
                for n in range(2):
                    nc.tensor.matmul(
                        psq[h][:, n * 512:(n + 1) * 512],
                        lhs, xc[:, n * 512:(n + 1) * 512],
                        start=(c == 0), stop=(c == KC - 1))
        for h in range(NH_CORE):
            nc.scalar.activation(qT[h][:], psq[h][:], Ident,
                                 bias=bq_t[:, h:h + 1])

    # ---- phase K: k^T = Wk_h @ mem^T (c-outer, by Tk side-half) ----------
    with ExitStack() as kctx:
        psk_pool = kctx.enter_context(
            tc.tile_pool(name="psk", bufs=8, space="PSUM"))
        mh_pool = kctx.enter_context(tc.tile_pool(name="mhk", bufs=3))
        wk_pool = kctx.enter_context(tc.tile_pool(name="wk", bufs=3))
        for s in range(2):
            psk = [psk_pool.tile([HD, 512], F32, name=f"psk{s}_{i}", tag="psk")
                   for i in range(8)]
            for c in range(KC):
                mc = mh_pool.tile([P, 1024], MM_DT, name="mck", tag="mck")
                nc.sync.dma_start(
                    out=mc[:],
                    in_=_mm(io["memT"][c * P:(c + 1) * P,
                                       s * 1024:(s + 1) * 1024]))
                wkc = wk_pool.tile([P, HG], MM_DT, name="wkc", tag="wkc")
                nc.sync.dma_start(out=wkc[:],
                                  in_=_mm(io["wkT"][c * P:(c + 1) * P, :]))
                for h in range(NH_CORE):
                    lhs = wkc[:, h * HD:(h + 1) * HD]
                    for n in range(2):
                        nc.tensor.matmul(
                            psk[h * 2 + n][:],
                            lhs, mc[:, n * 512:(n + 1) * 512],
                            start=(c == 0), stop=(c == KC - 1))
            for h in range(NH_CORE):
                for n in range(2):
                    col = s * 1024 + n * 512
                    nc.scalar.activation(kT[h][:, col:col + 512],
                                         psk[h * 2 + n][:], Ident,
                                         bias=bk_t[:, h:h + 1])

    # ---- phase V: v natural [Tk, 4*97] (c-outer, by Tk side-half) --------
    with ExitStack() as vctx:
        psv_pool = vctx.enter_context(
            tc.tile_pool(name="psv", bufs=8, space="PSUM"))
        mv_pool = vctx.enter_context(tc.tile_pool(name="mhv", bufs=3))
        wv_pool = vctx.enter_context(tc.tile_pool(name="wv", bufs=3))
        for s in range(2):
            psv = [psv_pool.tile([P, HG], F32, name=f"psv{s}_{i}", tag="psv")
                   for i in range(8)]
            for c in range(KC):
                mc = mv_pool.tile([P, 1024], MM_DT, name="mcv", tag="mcv")
                nc.sync.dma_start(
                    out=mc[:],
                    in_=_mm(io["memT"][c * P:(c + 1) * P,
                                       s * 1024:(s + 1) * 1024]))
                wvc = wv_pool.tile([P, HG], MM_DT, name="wvc", tag="wvc")
                nc.sync.dma_start(out=wvc[:],
                                  in_=_mm(io["wvT"][c * P:(c + 1) * P, :]))
                for ml in range(8):
                    nc.tensor.matmul(
                        psv[ml][:],
                        mc[:, ml * P:(ml + 1) * P], wvc[:],
                        start=(c == 0), stop=(c == KC - 1))
            for ml in range(8):
                mg = s * 8 + ml
                dst = vst[mg].rearrange("p (h c) -> p h c", c=VW)[:, :, 0:HD]
                src = psv[ml].rearrange("p (h c) -> p h c", c=HD)
                nc.vector.tensor_copy(dst, src)

    # ---- attention era ---------------------------------------------------
    acc_ps = ctx.enter_context(tc.tile_pool(name="acc_ps", bufs=2, space="PSUM"))
    s_ps = ctx.enter_context(tc.tile_pool(name="s_ps", bufs=2, space="PSUM"))
    pv_ps = ctx.enter_context(tc.tile_pool(name="pv_ps", bufs=2, space="PSUM"))
    p_pool = ctx.enter_context(tc.tile_pool(name="p_pool", bufs=3))
    tmp_pool = ctx.enter_context(tc.tile_pool(name="tmp_pool", bufs=3))
    aout_pool = ctx.enter_context(tc.tile_pool(name="aout_pool", bufs=2))
    aN_pool = ctx.enter_context(tc.tile_pool(name="aN_pool", bufs=1))
    ot_pool = ctx.enter_context(tc.tile_pool(name="ot_pool", bufs=3))

    aoutN = [aN_pool.tile([HD, TQ], MM_DT, name=f"aoutN{h}", tag=f"aoutN{h}")
             for h in range(NH_CORE)]
    wo_t = []
    for h in range(NH_CORE):
        w = const.tile([HD, D], MM_DT, name=f"wo_t{h}", tag=f"wo_t{h}")
        nc.sync.dma_start(out=w[:], in_=_mm(io["woT"][h * HD:(h + 1) * HD, :]))
        wo_t.append(w)

    def rope(dst, cE, sE, width):
        """In-place-free RoPE on dst [HD, width] given tables sliced to width."""
        for s in range(width // 1024):
            sl = slice(s * 1024, (s + 1) * 1024)
            sw = s_ps.tile([HD, 1024], F32, name="sw", tag="s")
            for n in range(2):
                nsl = slice(s * 1024 + n * 512, s * 1024 + (n + 1) * 512)
                nc.tensor.matmul(sw[:, n * 512:(n + 1) * 512],
                                 perm_t[:], dst[:, nsl])
            t1 = tmp_pool.tile([HD, 1024], F32, name="t1", tag="tmp")
            t2 = tmp_pool.tile([HD, 1024], F32, name="t2", tag="tmp")
            nc.vector.tensor_mul(t1[:], dst[:, sl], cE[:, sl])
            nc.vector.tensor_mul(t2[:], sw[:], sE[:, sl])
            nc.vector.tensor_add(dst[:, sl], t1[:], t2[:])

    for h in range(NH_CORE):
        rope(qT[h], cq_t, sq_t, TQ)
        rope(kT[h], ck_t, sk_t, TK)

        pv0 = pv_ps.tile([VW, 512], F32, name=f"pv{h}0", tag="pv")
        pv1 = pv_ps.tile([VW, 512], F32, name=f"pv{h}1", tag="pv")
        for kc in range(TK // P):
            st = s_ps.tile([P, TQ], F32, name="st", tag="s")
            lhs = kT[h][:, kc * P:(kc + 1) * P]
            nc.tensor.matmul(st[:, 0:512], lhs, qT[h][:, 0:512])
            nc.tensor.matmul(st[:, 512:1024], lhs, qT[h][:, 512:1024])
            pt = p_pool.tile([P, TQ], MM_DT, name="pt", tag="pt")
            nc.scalar.activation(pt[:], st[:], Exp, scale=SCALE)
            vl = vst[kc][:, h * VW:(h + 1) * VW]
            first, last = (kc == 0), (kc == TK // P - 1)
            nc.tensor.matmul(pv0[:], vl, pt[:, 0:512],
                             start=first, stop=last)
            nc.tensor.matmul(pv1[:], vl, pt[:, 512:1024],
                             start=first, stop=last)
        aout = aout_pool.tile([VW, TQ], F32, name="aout", tag="aout")
        nc.vector.tensor_copy(aout[:, 0:512], pv0[:])
        nc.vector.tensor_copy(aout[:, 512:1024], pv1[:])
        rec = tmp_pool.tile([1, TQ], MM_DT, name="rec", tag="rec")
        with nc.allow_low_precision(reason="f32r is fp32-width storage"):
            nc.vector.reciprocal(rec[:], aout[HD:HD + 1, :])
        rb = s_ps.tile([P, TQ], F32, name="rb", tag="s")
        for n in range(2):
            nc.tensor.matmul(rb[:, n * 512:(n + 1) * 512],
                             ones1_t[:],
                             rec[:, n * 512:(n + 1) * 512])
        nc.vector.tensor_mul(aoutN[h][:], aout[0:HD, :], rb[0:HD, :])

    # ---- output projection ----------------------------------------------
    for m in range(MTILES):
        ot = ot_pool.tile([P, TQ], F32, name="ot", tag="ot")
        for n in range(2):
            po = acc_ps.tile([P, 512], F32, name="po", tag="acc")
            for h in range(NH_CORE):
                nc.tensor.matmul(po[:],
                                 wo_t[h][:, m * P:(m + 1) * P],
                                 aoutN[h][:, n * 512:(n + 1) * 512],
                                 start=(h == 0), stop=(h == NH_CORE - 1))
            if n == 0:
                nc.vector.tensor_copy(ot[:, 0:512], po[:])
            else:
                nc.scalar.copy(ot[:, 512:1024], po[:])
        nc.sync.dma_start(out=oT[m * P:(m + 1) * P, :], in_=ot[:])


# ---------------------------------------------------------------- host side
def _rope_tables(coords, T):
    """Feature-major cos/sin tables [HD, T] with the sign fold.

    Row j < 48 of the rotated output is q[j]*cos_j - q[j+48]*sin_j and row
    j >= 48 is q[j]*cos_{j-48} + q[j-48]*sin_{j-48}; the device computes
    rot = q * cE + swap(q) * sE with swap(q)[j] = q[(j+48) % 96].
    """
    coords = np.asarray(coords, np.float32)
    inv_freq = (1.0 / (ROPE_BASE ** (np.arange(FREQ_PER_AXIS, dtype=np.float32)
                                     / FREQ_PER_AXIS))).astype(np.float32)
    ang = coords[:, :, None] * inv_freq[None, None, :]   # [T, 3, 16]
    ang = ang.reshape(T, ROPE_HALF)                      # [T, 48]
    sin = np.sin(ang).astype(np.float32).T               # [48, T]
    cos = np.cos(ang).astype(np.float32).T
    cE = np.concatenate([cos, cos], axis=0)              # [96, T]
    sE = np.concatenate([-sin, sin], axis=0)
    return np.ascontiguousarray(cE), np.ascontiguousarray(sE)


def _make_in_maps(inputs):
    x = np.asarray(inputs["x"], np.float32)
    memory = np.asarray(inputs["memory"], np.float32)
    qc = np.asarray(inputs["query_coords"], np.float32)
    mc = np.asarray(inputs["memory_coords"], np.float32)
    Wq = np.asarray(inputs["Wq"], np.float32)
    Wk = np.asarray(inputs["Wk"], np.float32)
    Wv = np.asarray(inputs["Wv"], np.float32)
    Wo = np.asarray(inputs["Wo"], np.float32)
    bq = np.asarray(inputs["bq"], np.float32)
    bk = np.asarray(inputs["bk"], np.float32)

    WqT = np.ascontiguousarray(Wq.T)   # [in, out]
    WkT = np.ascontiguousarray(Wk.T)
    WvT = np.ascontiguousarray(Wv.T)
    WoT = np.ascontiguousarray(Wo.T)

    perm = np.zeros((HD, HD), np.float32)   # lhsT: perm[j, i] = [j == (i+48)%96]
    for i in range(HD):
        perm[(i + ROPE_HALF) % HD, i] = 1.0

    per_batch = []
    for b in range(B):
        cqE, sqE = _rope_tables(qc[b], TQ)
        ckE, skE = _rope_tables(mc[b], TK)
        per_batch.append({
            "xT": np.ascontiguousarray(x[b].T),
            "memT": np.ascontiguousarray(memory[b].T),
            "cqE": cqE, "sqE": sqE, "ckE": ckE, "skE": skE,
        })

    in_maps = []
    for core in range(N_CORES):
        b, g = divmod(core, NH_CORE)
        sl = slice(g * HG, (g + 1) * HG)
        m = dict(per_batch[b])
        m["wqT"] = np.ascontiguousarray(WqT[:, sl])
        m["wkT"] = np.ascontiguousarray(WkT[:, sl])
        m["wvT"] = np.ascontiguousarray(WvT[:, sl])
        m["woT"] = np.ascontiguousarray(WoT[sl, :])
        m["bq4"] = np.ascontiguousarray(bq[sl].reshape(NH_CORE, HD).T)
        m["bk4"] = np.ascontiguousarray(bk[sl].reshape(NH_CORE, HD).T)
        m["perm"] = perm
        m["ones1"] = np.ones((1, 128), np.float32)
        m["ones4"] = np.ones((128, NH_CORE), np.float32)
        in_maps.append(m)
    return in_maps


def _assemble(results, inputs):
    Wo = np.asarray(inputs["Wo"], np.float32)
    bv = np.asarray(inputs["bv"], np.float32)
    bo = np.asarray(inputs["bo"], np.float32)
    cvec = (bv @ Wo.T + bo).astype(np.float32)   # exact: attn rows sum to 1
    out = np.empty((B, TQ, D), np.float32)
    for b in range(B):
        acc = np.zeros((D, TQ), np.float64)
        for g in range(NH_CORE):
            acc += results[b * NH_CORE + g]["oT"]
        out[b] = acc.T.astype(np.float32) + cvec
    return out


_NC_CACHE = None


def _get_nc():
    global _NC_CACHE
    if _NC_CACHE is None:
        _NC_CACHE = _build_nc()
    return _NC_CACHE


def kernel(**inputs) -> np.ndarray:
    nc = _get_nc()
    in_maps = _make_in_maps(inputs)
    res = run_bass_kernel_spmd(nc, in_maps, list(range(N_CORES)))
    return _assemble(res.results, inputs)


# revision 22
# speedup vs baseline: 1.0835x; 1.0835x over previous
"""Trainium2 Bass kernel for a cross-attention block with 3D-coordinate RoPE.

Module: q/k/v projections of x [B,Tq,D] against memory [B,Tk,D], 3D-coord
rotary embedding on q/k, softmax(q k^T / sqrt(Hd)) v, output projection.
B=2, Tq=1024, Tk=2048, D=1536, 16 heads x 96.

Sharding: 8 cores = (2 batches) x (4 head-groups of 4 heads). Each core
computes its heads end-to-end plus a partial output projection; the host
sums the 4 partials per batch. Biases bv/bo are folded in on the host
(attention rows sum to one), bq/bk are added on-device during PSUM
eviction.

Layout: feature-major ("transposed") on device. Scores are computed
transposed (S^T = k q^T) so the PV matmul needs no on-chip transposes;
softmax denominators come from a ones-column appended to v; the
per-query normalization is broadcast across partitions with a K=1
ones-vector matmul + full-lane approximate reciprocal.

Matmul dtype is selectable via KMM_DTYPE in {bf16, f32r, f32}; logits,
softmax, denominators and RoPE trig tables stay fp32 in all modes.
"""

import os
import sys

sys.path.insert(0, "/opt/trn_rl_repo")

import numpy as np
import ml_dtypes
from contextlib import ExitStack

import concourse.bass as bass
import concourse.tile as tile
from concourse import bacc, mybir
from concourse.bass_utils import run_bass_kernel_spmd

# ---------------------------------------------------------------- constants
B = 2
TQ = 1024
TK = 2048
D = 1536
NH = 16
HD = 96
ROPE_HALF = HD // 2           # 48
FREQ_PER_AXIS = ROPE_HALF // 3  # 16
ROPE_BASE = 10000.0
NH_CORE = 4                   # heads per core
HG = NH_CORE * HD             # 384 features per core
KC = D // 128                 # 12 contraction chunks
MTILES = D // 128             # 12 output-row tiles of the o-projection
SCALE = 1.0 / float(np.sqrt(HD))
N_CORES = 8
VW = HD + 1                   # 97: head-dim + ones column

F32 = mybir.dt.float32

_MM_DT_NAME = os.environ.get("KMM_DTYPE", "mixed")
_DT = {"f32r": mybir.dt.float32r, "f32": mybir.dt.float32,
       "bf16": mybir.dt.bfloat16}
_NP = {"f32r": np.float32, "f32": np.float32, "bf16": ml_dtypes.bfloat16}
if _MM_DT_NAME == "mixed":          # q/k chain fp32r, v/attn-weight/out bf16
    _QK_NAME, _PV_NAME = "f32r", "bf16"
else:
    _QK_NAME = _PV_NAME = _MM_DT_NAME
QK_DT, QK_NP = _DT[_QK_NAME], _NP[_QK_NAME]
PV_DT, PV_NP = _DT[_PV_NAME], _NP[_PV_NAME]
SPLIT_MEM = _QK_NAME != _PV_NAME    # ship memory twice (per-dtype) if mixed


# ---------------------------------------------------------------- bass build
def _build_nc():
    nc = bacc.Bacc(trn_type="TRN2", target_bir_lowering=False, debug=False)

    io = {}
    def dram_in(name, shape, dt):
        io[name] = nc.dram_tensor(name, list(shape), dt, kind="ExternalInput").ap()
    dram_in("xT", [D, TQ], QK_DT)
    dram_in("memT", [D, TK], QK_DT)
    if SPLIT_MEM:
        dram_in("memTv", [D, TK], PV_DT)
    dram_in("wqT", [D, HG], QK_DT)  # columns of Wq^T for this head group
    dram_in("wkT", [D, HG], QK_DT)
    dram_in("wvT", [D, HG], PV_DT)
    dram_in("woT", [HG, D], PV_DT)  # rows of Wo^T for this head group
    dram_in("bq4", [HD, NH_CORE], F32)
    dram_in("bk4", [HD, NH_CORE], F32)
    dram_in("cqE", [HD, TQ], F32)  # cos table, feature-major, q side
    dram_in("sqE", [HD, TQ], F32)  # sign-folded sin table, q side
    dram_in("ckE", [HD, TK], F32)
    dram_in("skE", [HD, TK], F32)
    dram_in("perm", [HD, HD], QK_DT)  # lhsT of the half-rotation swap
    dram_in("ones1", [1, 128], PV_DT)
    dram_in("ones4", [128, NH_CORE], PV_DT)
    oT = nc.dram_tensor("oT", [D, TQ], F32, kind="ExternalOutput").ap()

    with tile.TileContext(nc) as tc, ExitStack() as ctx:
        _body(ctx, tc, io, oT)
    nc.compile()
    return nc


def _body(ctx, tc, io, oT):
    nc = tc.nc
    P = 128
    Exp = mybir.ActivationFunctionType.Exp
    Ident = mybir.ActivationFunctionType.Identity

    const = ctx.enter_context(tc.tile_pool(name="const", bufs=1))
    resident = ctx.enter_context(tc.tile_pool(name="resident", bufs=1))

    # ---- constants -------------------------------------------------------
    perm_t = const.tile([HD, HD], QK_DT, name="perm_t")
    nc.sync.dma_start(out=perm_t[:], in_=io["perm"][:])
    ones1_t = const.tile([1, P], PV_DT, name="ones1_t")
    nc.sync.dma_start(out=ones1_t[:], in_=io["ones1"][:])
    bq_t = const.tile([HD, NH_CORE], F32, name="bq_t")
    nc.sync.dma_start(out=bq_t[:], in_=io["bq4"][:])
    bk_t = const.tile([HD, NH_CORE], F32, name="bk_t")
    nc.sync.dma_start(out=bk_t[:], in_=io["bk4"][:])
    cq_t = const.tile([HD, TQ], F32, name="cq_t")
    nc.sync.dma_start(out=cq_t[:], in_=io["cqE"][:])
    sq_t = const.tile([HD, TQ], F32, name="sq_t")
    nc.sync.dma_start(out=sq_t[:], in_=io["sqE"][:])
    ck_t = const.tile([HD, TK], F32, name="ck_t")
    nc.sync.dma_start(out=ck_t[:], in_=io["ckE"][:])
    sk_t = const.tile([HD, TK], F32, name="sk_t")
    nc.sync.dma_start(out=sk_t[:], in_=io["skE"][:])

    # ---- long-lived results ---------------------------------------------
    qT = [resident.tile([HD, TQ], QK_DT, name=f"qT{h}", tag=f"qT{h}")
          for h in range(NH_CORE)]
    kT = [resident.tile([HD, TK], QK_DT, name=f"kT{h}", tag=f"kT{h}")
          for h in range(NH_CORE)]
    vst = [resident.tile([P, NH_CORE * VW], PV_DT, name=f"vst{m}", tag=f"vst{m}")
           for m in range(TK // P)]
    # ones columns of v (one strided DMA per m-tile)
    for m in range(TK // P):
        ones_cols = vst[m].rearrange("p (h c) -> p h c", c=VW)[:, :, HD:HD + 1]
        nc.sync.dma_start(out=ones_cols,
                          in_=io["ones4"][:].rearrange("p (h c) -> p h c", c=1))

    # k/v weights, resident through phases K and V
    wk_all = const.tile([P, KC, HG], QK_DT, name="wk_all")
    nc.sync.dma_start(out=wk_all[:],
                      in_=io["wkT"][:].rearrange("(c p) n -> p c n", p=P))
    wv_all = const.tile([P, KC, HG], PV_DT, name="wv_all")
    nc.sync.dma_start(out=wv_all[:],
                      in_=io["wvT"][:].rearrange("(c p) n -> p c n", p=P))

    # ---- phase Q: q^T = Wq_h @ x^T (c-outer, 8 psum banks) ---------------
    with ExitStack() as qctx:
        psq_pool = qctx.enter_context(
            tc.tile_pool(name="psq", bufs=NH_CORE, space="PSUM"))
        xq_pool = qctx.enter_context(tc.tile_pool(name="xq", bufs=3))
        wq_pool = qctx.enter_context(tc.tile_pool(name="wq", bufs=1))
        wq_all = wq_pool.tile([P, KC, HG], QK_DT, name="wq_all")
        nc.sync.dma_start(out=wq_all[:],
                          in_=io["wqT"][:].rearrange("(c p) n -> p c n", p=P))
        psq = [psq_pool.tile([HD, TQ], F32, name=f"psq{h}", tag="psq")
               for h in range(NH_CORE)]
        for c in range(KC):
            xc = xq_pool.tile([P, TQ], QK_DT, name="xc", tag="xc")
            nc.sync.dma_start(out=xc[:], in_=io["xT"][c * P:(c + 1) * P, :])
            for h in range(NH_CORE):
                lhs = wq_all[:, c, h * HD:(h + 1) * HD]
                for n in range(2):
                    nc.tensor.matmul(
                        psq[h][:, n * 512:(n + 1) * 512],
                        lhs, xc[:, n * 512:(n + 1) * 512],
                        start=(c == 0), stop=(c == KC - 1))
        for h in range(NH_CORE):
            nc.scalar.activation(qT[h][:], psq[h][:], Ident,
                                 bias=bq_t[:, h:h + 1])

    # psum pools for the rest of the kernel: s_ps lives from here to the
    # end (4 banks) so RoPE swap matmuls can overlap phases K and V.
    s_ps = ctx.enter_context(tc.tile_pool(name="s_ps", bufs=2, space="PSUM"))
    tmp_pool = ctx.enter_context(tc.tile_pool(name="tmp_pool", bufs=3))
    p_pool = ctx.enter_context(tc.tile_pool(name="p_pool", bufs=3))

    def rope(dst, cE, sE, lo, width):
        """RoPE on dst[:, lo:lo+width] (width multiple of 1024)."""
        for s in range(width // 1024):
            sl = slice(lo + s * 1024, lo + (s + 1) * 1024)
            sw = s_ps.tile([HD, 1024], F32, name="sw", tag="s")
            for n in range(2):
                nsl = slice(lo + s * 1024 + n * 512, lo + s * 1024 + (n + 1) * 512)
                nc.tensor.matmul(sw[:, n * 512:(n + 1) * 512],
                                 perm_t[:], dst[:, nsl])
            t1 = tmp_pool.tile([HD, 1024], F32, name="t1", tag="tmp")
            t2 = tmp_pool.tile([HD, 1024], F32, name="t2", tag="tmp")
            nc.vector.tensor_mul(t1[:], dst[:, sl], cE[:, sl])
            nc.vector.tensor_mul(t2[:], sw[:], sE[:, sl])
            nc.vector.tensor_add(dst[:, sl], t1[:], t2[:])

    # RoPE on q can overlap phase K below
    for h in range(NH_CORE):
        rope(qT[h], cq_t, sq_t, 0, TQ)

    # ---- phase K: k^T = Wk_h @ mem^T (c-outer, by Tk quarter) ------------
    with ExitStack() as kctx:
        psk_pool = kctx.enter_context(
            tc.tile_pool(name="psk", bufs=4, space="PSUM"))
        mh_pool = kctx.enter_context(tc.tile_pool(name="mhk", bufs=3))
        for q4 in range(4):
            col = q4 * 512
            psk = [psk_pool.tile([HD, 512], F32, name=f"psk{q4}_{h}", tag="psk")
                   for h in range(NH_CORE)]
            for c in range(KC):
                mc = mh_pool.tile([P, 512], QK_DT, name="mck", tag="mck")
                nc.sync.dma_start(
                    out=mc[:], in_=io["memT"][c * P:(c + 1) * P, col:col + 512])
                for h in range(NH_CORE):
                    nc.tensor.matmul(
                        psk[h][:], wk_all[:, c, h * HD:(h + 1) * HD], mc[:],
                        start=(c == 0), stop=(c == KC - 1))
            for h in range(NH_CORE):
                nc.scalar.activation(kT[h][:, col:col + 512],
                                     psk[h][:], Ident, bias=bk_t[:, h:h + 1])
            if q4 % 2 == 1:   # a full 1024-wide half is done -> rotate it
                for h in range(NH_CORE):
                    rope(kT[h], ck_t, sk_t, (q4 // 2) * 1024, 1024)

    # ---- phase V: v natural [Tk, 4*97] (c-outer, by Tk quarter) ----------
    with ExitStack() as vctx:
        psv_pool = vctx.enter_context(
            tc.tile_pool(name="psv", bufs=4, space="PSUM"))
        mv_pool = vctx.enter_context(tc.tile_pool(name="mhv", bufs=3))
        for q4 in range(4):
            col = q4 * 512
            psv = [psv_pool.tile([P, HG], F32, name=f"psv{q4}_{i}", tag="psv")
                   for i in range(4)]
            for c in range(KC):
                mc = mv_pool.tile([P, 512], PV_DT, name="mcv", tag="mcv")
                mem_v = io["memTv"] if SPLIT_MEM else io["memT"]
                nc.sync.dma_start(
                    out=mc[:], in_=mem_v[c * P:(c + 1) * P, col:col + 512])
                for ml in range(4):
                    nc.tensor.matmul(
                        psv[ml][:], mc[:, ml * P:(ml + 1) * P], wv_all[:, c, :],
                        start=(c == 0), stop=(c == KC - 1))
            for ml in range(4):
                mg = q4 * 4 + ml
                dst = vst[mg].rearrange("p (h c) -> p h c", c=VW)[:, :, 0:HD]
                src = psv[ml].rearrange("p (h c) -> p h c", c=HD)
                nc.vector.tensor_copy(dst, src)
    # ---- attention -------------------------------------------------------
    acc_ps = ctx.enter_context(tc.tile_pool(name="acc_ps", bufs=2, space="PSUM"))
    pv_ps = ctx.enter_context(tc.tile_pool(name="pv_ps", bufs=2, space="PSUM"))
    aout_pool = ctx.enter_context(tc.tile_pool(name="aout_pool", bufs=2))
    aN_pool = ctx.enter_context(tc.tile_pool(name="aN_pool", bufs=1))
    ot_pool = ctx.enter_context(tc.tile_pool(name="ot_pool", bufs=3))

    aoutN = [aN_pool.tile([HD, TQ], PV_DT, name=f"aoutN{h}", tag=f"aoutN{h}")
             for h in range(NH_CORE)]
    wo_t = []
    for h in range(NH_CORE):
        w = const.tile([HD, D], PV_DT, name=f"wo_t{h}", tag=f"wo_t{h}")
        nc.sync.dma_start(out=w[:], in_=io["woT"][h * HD:(h + 1) * HD, :])
        wo_t.append(w)

    for h in range(NH_CORE):
        pv0 = pv_ps.tile([VW, 512], F32, name=f"pv{h}0", tag="pv")
        pv1 = pv_ps.tile([VW, 512], F32, name=f"pv{h}1", tag="pv")
        for kc in range(TK // P):
            st = s_ps.tile([P, TQ], F32, name="st", tag="s")
            lhs = kT[h][:, kc * P:(kc + 1) * P]
            nc.tensor.matmul(st[:, 0:512], lhs, qT[h][:, 0:512])
            nc.tensor.matmul(st[:, 512:1024], lhs, qT[h][:, 512:1024])
            pt = p_pool.tile([P, TQ], PV_DT, name="pt", tag="pt")
            nc.scalar.activation(pt[:], st[:], Exp, scale=SCALE)
            vl = vst[kc][:, h * VW:(h + 1) * VW]
            first, last = (kc == 0), (kc == TK // P - 1)
            nc.tensor.matmul(pv0[:], vl, pt[:, 0:512], start=first, stop=last)
            nc.tensor.matmul(pv1[:], vl, pt[:, 512:1024], start=first, stop=last)
        aout = aout_pool.tile([VW, TQ], PV_DT, name="aout", tag="aout")
        nc.vector.tensor_copy(aout[:, 0:512], pv0[:])
        nc.vector.tensor_copy(aout[:, 512:1024], pv1[:])
        # broadcast the denominator row across partitions (K=1 matmul),
        # then full-lane approximate reciprocal and normalize
        den1 = tmp_pool.tile([1, TQ], PV_DT, name="den1", tag="den1")
        nc.vector.tensor_copy(den1[:], aout[HD:HD + 1, :])
        denB = s_ps.tile([P, TQ], F32, name="denB", tag="s")
        for n in range(2):
            nc.tensor.matmul(denB[:, n * 512:(n + 1) * 512], ones1_t[:],
                             den1[:, n * 512:(n + 1) * 512])
        recB = tmp_pool.tile([HD, TQ], F32, name="recB", tag="tmp")
        nc.vector.reciprocal_approx_fast(out=recB[:], in_=denB[0:HD, :])
        nc.vector.tensor_mul(aoutN[h][:], aout[0:HD, :], recB[:])

    # ---- output projection ----------------------------------------------
    for m in range(MTILES):
        ot = ot_pool.tile([P, TQ], F32, name="ot", tag="ot")
        for n in range(2):
            po = acc_ps.tile([P, 512], F32, name="po", tag="acc")
            for h in range(NH_CORE):
                nc.tensor.matmul(po[:],
                                 wo_t[h][:, m * P:(m + 1) * P],
                                 aoutN[h][:, n * 512:(n + 1) * 512],
                                 start=(h == 0), stop=(h == NH_CORE - 1))
            if n == 0:
                nc.vector.tensor_copy(ot[:, 0:512], po[:])
            else:
                nc.scalar.copy(ot[:, 512:1024], po[:])
        nc.sync.dma_start(out=oT[m * P:(m + 1) * P, :], in_=ot[:])


# ---------------------------------------------------------------- host side
def _rope_tables(coords, T):
    """Feature-major cos/sin tables [HD, T] with the sign fold.

    Row j < 48 of the rotated output is q[j]*cos_j - q[j+48]*sin_j and row
    j >= 48 is q[j]*cos_{j-48} + q[j-48]*sin_{j-48}; the device computes
    rot = q * cE + swap(q) * sE with swap(q)[j] = q[(j+48) % 96].
    """
    coords = np.asarray(coords, np.float32)
    inv_freq = (1.0 / (ROPE_BASE ** (np.arange(FREQ_PER_AXIS, dtype=np.float32)
                                     / FREQ_PER_AXIS))).astype(np.float32)
    ang = coords[:, :, None] * inv_freq[None, None, :]   # [T, 3, 16]
    ang = ang.reshape(T, ROPE_HALF)                      # [T, 48]
    sin = np.sin(ang).astype(np.float32).T               # [48, T]
    cos = np.cos(ang).astype(np.float32).T
    cE = np.concatenate([cos, cos], axis=0)              # [96, T]
    sE = np.concatenate([-sin, sin], axis=0)
    return np.ascontiguousarray(cE), np.ascontiguousarray(sE)


def _make_in_maps(inputs):
    x = np.asarray(inputs["x"], np.float32)
    memory = np.asarray(inputs["memory"], np.float32)
    qc = np.asarray(inputs["query_coords"], np.float32)
    mc = np.asarray(inputs["memory_coords"], np.float32)
    Wq = np.asarray(inputs["Wq"], np.float32)
    Wk = np.asarray(inputs["Wk"], np.float32)
    Wv = np.asarray(inputs["Wv"], np.float32)
    Wo = np.asarray(inputs["Wo"], np.float32)
    bq = np.asarray(inputs["bq"], np.float32)
    bk = np.asarray(inputs["bk"], np.float32)

    WqT = np.ascontiguousarray(Wq.T).astype(QK_NP)   # [in, out]
    WkT = np.ascontiguousarray(Wk.T).astype(QK_NP)
    WvT = np.ascontiguousarray(Wv.T).astype(PV_NP)
    WoT = np.ascontiguousarray(Wo.T).astype(PV_NP)

    perm = np.zeros((HD, HD), QK_NP)   # lhsT: perm[j, i] = [j == (i+48)%96]
    for i in range(HD):
        perm[(i + ROPE_HALF) % HD, i] = 1.0

    per_batch = []
    for b in range(B):
        cqE, sqE = _rope_tables(qc[b], TQ)
        ckE, skE = _rope_tables(mc[b], TK)
        entry = {
            "xT": np.ascontiguousarray(x[b].T).astype(QK_NP),
            "memT": np.ascontiguousarray(memory[b].T).astype(QK_NP),
            "cqE": cqE, "sqE": sqE, "ckE": ckE, "skE": skE,
        }
        if SPLIT_MEM:
            entry["memTv"] = np.ascontiguousarray(memory[b].T).astype(PV_NP)
        per_batch.append(entry)

    in_maps = []
    for core in range(N_CORES):
        b, g = divmod(core, NH_CORE)
        sl = slice(g * HG, (g + 1) * HG)
        m = dict(per_batch[b])
        m["wqT"] = np.ascontiguousarray(WqT[:, sl])
        m["wkT"] = np.ascontiguousarray(WkT[:, sl])
        m["wvT"] = np.ascontiguousarray(WvT[:, sl])
        m["woT"] = np.ascontiguousarray(WoT[sl, :])
        m["bq4"] = np.ascontiguousarray(bq[sl].reshape(NH_CORE, HD).T)
        m["bk4"] = np.ascontiguousarray(bk[sl].reshape(NH_CORE, HD).T)
        m["perm"] = perm
        m["ones1"] = np.ones((1, 128), PV_NP)
        m["ones4"] = np.ones((128, NH_CORE), PV_NP)
        in_maps.append(m)
    return in_maps


def _assemble(results, inputs):
    Wo = np.asarray(inputs["Wo"], np.float32)
    bv = np.asarray(inputs["bv"], np.float32)
    bo = np.asarray(inputs["bo"], np.float32)
    cvec = (bv @ Wo.T + bo).astype(np.float32)   # exact: attn rows sum to 1
    out = np.empty((B, TQ, D), np.float32)
    for b in range(B):
        acc = np.zeros((D, TQ), np.float64)
        for g in range(NH_CORE):
            acc += results[b * NH_CORE + g]["oT"]
        out[b] = acc.T.astype(np.float32) + cvec
    return out


_NC_CACHE = None


def _get_nc():
    global _NC_CACHE
    if _NC_CACHE is None:
        _NC_CACHE = _build_nc()
    return _NC_CACHE


def kernel(**inputs) -> np.ndarray:
    nc = _get_nc()
    in_maps = _make_in_maps(inputs)
    res = run_bass_kernel_spmd(nc, in_maps, list(range(N_CORES)))
    return _assemble(res.results, inputs)


# revision 24
# speedup vs baseline: 1.3102x; 1.2093x over previous
"""Trainium2 Bass kernel for a cross-attention block with 3D-coordinate RoPE.

Module: q/k/v projections of x [B,Tq,D] against memory [B,Tk,D], 3D-coord
rotary embedding on q/k, softmax(q k^T / sqrt(Hd)) v, output projection.
B=2, Tq=1024, Tk=2048, D=1536, 16 heads x 96.

Sharding: 8 cores = (2 batches) x (4 head-groups of 4 heads). Each core
computes its heads end-to-end plus a partial output projection; the host
sums the 4 partials per batch. Biases bv/bo are folded in on the host
(attention rows sum to one), bq/bk are added on-device during PSUM
eviction.

Layout: feature-major ("transposed") on device. Scores are computed
transposed (S^T = k q^T) so the PV matmul needs no on-chip transposes;
softmax denominators come from a ones-column appended to v; the
per-query normalization is broadcast across partitions with a K=1
ones-vector matmul + full-lane approximate reciprocal.

Matmul dtype is selectable via KMM_DTYPE in {bf16, f32r, f32}; logits,
softmax, denominators and RoPE trig tables stay fp32 in all modes.
"""

import os
import sys

sys.path.insert(0, "/opt/trn_rl_repo")

import numpy as np
import ml_dtypes
from contextlib import ExitStack

import concourse.bass as bass
import concourse.tile as tile
from concourse import bacc, mybir
from concourse.bass_utils import run_bass_kernel_spmd

# ---------------------------------------------------------------- constants
B = 2
TQ = 1024
TK = 2048
D = 1536
NH = 16
HD = 96
ROPE_HALF = HD // 2           # 48
FREQ_PER_AXIS = ROPE_HALF // 3  # 16
ROPE_BASE = 10000.0
NH_CORE = 4                   # heads per core
HG = NH_CORE * HD             # 384 features per core
KC = D // 128                 # 12 contraction chunks
MTILES = D // 128             # 12 output-row tiles of the o-projection
SCALE = 1.0 / float(np.sqrt(HD))
N_CORES = 8
VW = HD + 1                   # 97: head-dim + ones column

F32 = mybir.dt.float32

_MM_DT_NAME = os.environ.get("KMM_DTYPE", "mixed")
_DT = {"f32r": mybir.dt.float32r, "f32": mybir.dt.float32,
       "bf16": mybir.dt.bfloat16}
_NP = {"f32r": np.float32, "f32": np.float32, "bf16": ml_dtypes.bfloat16}
if _MM_DT_NAME == "mixed":          # q/k chain fp32r, v/attn-weight/out bf16
    _QK_NAME, _PV_NAME = "f32r", "bf16"
else:
    _QK_NAME = _PV_NAME = _MM_DT_NAME
QK_DT, QK_NP = _DT[_QK_NAME], _NP[_QK_NAME]
PV_DT, PV_NP = _DT[_PV_NAME], _NP[_PV_NAME]
SPLIT_MEM = _QK_NAME != _PV_NAME    # ship memory twice (per-dtype) if mixed


# ---------------------------------------------------------------- bass build
def _build_nc():
    nc = bacc.Bacc(trn_type="TRN2", target_bir_lowering=False, debug=False)

    io = {}
    def dram_in(name, shape, dt):
        io[name] = nc.dram_tensor(name, list(shape), dt, kind="ExternalInput").ap()
    dram_in("xT", [D, TQ], QK_DT)
    dram_in("memT", [D, TK], QK_DT)
    if SPLIT_MEM:
        dram_in("memTv", [D, TK], PV_DT)
    dram_in("wqT", [D, HG], QK_DT)  # columns of Wq^T for this head group
    dram_in("wkT", [D, HG], QK_DT)
    dram_in("wvT", [D, HG], PV_DT)
    dram_in("woT", [HG, D], PV_DT)  # rows of Wo^T for this head group
    dram_in("bq4", [HD, NH_CORE], F32)
    dram_in("bk4", [HD, NH_CORE], F32)
    dram_in("cqE", [HD, TQ], F32)  # cos table, feature-major, q side
    dram_in("sqE", [HD, TQ], F32)  # sign-folded sin table, q side
    dram_in("ckE", [HD, TK], F32)
    dram_in("skE", [HD, TK], F32)
    dram_in("perm", [HD, HD], QK_DT)  # lhsT of the half-rotation swap
    dram_in("ones1", [1, 128], PV_DT)
    dram_in("ones4", [128, NH_CORE], PV_DT)
    oT = nc.dram_tensor("oT", [D, TQ], F32, kind="ExternalOutput").ap()

    with tile.TileContext(nc) as tc, ExitStack() as ctx:
        _body(ctx, tc, io, oT)
    nc.compile()
    return nc


def _body(ctx, tc, io, oT):
    nc = tc.nc
    P = 128
    NKC = TK // P
    Exp = mybir.ActivationFunctionType.Exp
    Ident = mybir.ActivationFunctionType.Identity

    const = ctx.enter_context(tc.tile_pool(name="const", bufs=1))
    resident = ctx.enter_context(tc.tile_pool(name="resident", bufs=1))

    # long-lived tiles (allocation only; DMAs are emitted just-in-time so
    # the HWDGE FIFO serves the critical path first)
    perm_t = const.tile([HD, HD], QK_DT, name="perm_t")
    ones1_t = const.tile([1, P], PV_DT, name="ones1_t")
    bq_t = const.tile([HD, NH_CORE], F32, name="bq_t")
    bk_t = const.tile([HD, NH_CORE], F32, name="bk_t")
    cq_t = const.tile([HD, TQ], F32, name="cq_t")
    sq_t = const.tile([HD, TQ], F32, name="sq_t")
    ck_t = const.tile([HD, TK], F32, name="ck_t")
    sk_t = const.tile([HD, TK], F32, name="sk_t")
    wk_all = const.tile([P, KC, HG], QK_DT, name="wk_all")
    wv_all = const.tile([P, KC, HG], PV_DT, name="wv_all")
    qT = [resident.tile([HD, TQ], QK_DT, name=f"qT{h}", tag=f"qT{h}")
          for h in range(NH_CORE)]
    kT = [resident.tile([HD, TK], QK_DT, name=f"kT{h}", tag=f"kT{h}")
          for h in range(NH_CORE)]
    vst = [resident.tile([P, NH_CORE * VW], PV_DT, name=f"vst{m}", tag=f"vst{m}")
           for m in range(NKC)]

    def load_w_chunks(dst, src_ap):
        for c in range(KC):
            nc.sync.dma_start(out=dst[:, c, :],
                              in_=src_ap[c * P:(c + 1) * P, :])

    # ---- phase Q: q^T = Wq_h @ x^T (c-outer, 8 psum banks) ---------------
    with ExitStack() as qctx:
        psq_pool = qctx.enter_context(
            tc.tile_pool(name="psq", bufs=NH_CORE, space="PSUM"))
        xq_pool = qctx.enter_context(tc.tile_pool(name="xq", bufs=3))
        wq_pool = qctx.enter_context(tc.tile_pool(name="wq", bufs=1))
        wq_all = wq_pool.tile([P, KC, HG], QK_DT, name="wq_all")
        psq = [psq_pool.tile([HD, TQ], F32, name=f"psq{h}", tag="psq")
               for h in range(NH_CORE)]
        for c in range(KC):
            nc.sync.dma_start(out=wq_all[:, c, :],
                              in_=io["wqT"][c * P:(c + 1) * P, :])
            xc = xq_pool.tile([P, TQ], QK_DT, name="xc", tag="xc")
            nc.sync.dma_start(out=xc[:], in_=io["xT"][c * P:(c + 1) * P, :])
            for h in range(NH_CORE):
                lhs = wq_all[:, c, h * HD:(h + 1) * HD]
                for n in range(2):
                    nc.tensor.matmul(
                        psq[h][:, n * 512:(n + 1) * 512],
                        lhs, xc[:, n * 512:(n + 1) * 512],
                        start=(c == 0), stop=(c == KC - 1))
        nc.sync.dma_start(out=bq_t[:], in_=io["bq4"][:])
        for h in range(NH_CORE):
            nc.scalar.activation(qT[h][:], psq[h][:], Ident,
                                 bias=bq_t[:, h:h + 1])

    # swap-psum pool for RoPE, alive only through phase K (LIFO with psk)
    tmp_pool = ctx.enter_context(tc.tile_pool(name="tmp_pool", bufs=3))
    p_pool = ctx.enter_context(tc.tile_pool(name="p_pool", bufs=4))
    sw_stack = ExitStack()
    sw_ps = sw_stack.enter_context(
        tc.tile_pool(name="sw_ps", bufs=2, space="PSUM"))

    def rope(dst, cE, sE, lo, width):
        """RoPE on dst[:, lo:lo+width] (width multiple of 1024)."""
        for s in range(width // 1024):
            sl = slice(lo + s * 1024, lo + (s + 1) * 1024)
            sw = sw_ps.tile([HD, 1024], F32, name="sw", tag="sw")
            for n in range(2):
                nsl = slice(lo + s * 1024 + n * 512, lo + s * 1024 + (n + 1) * 512)
                nc.tensor.matmul(sw[:, n * 512:(n + 1) * 512],
                                 perm_t[:], dst[:, nsl])
            t1 = tmp_pool.tile([HD, 1024], F32, name="t1", tag="tmp")
            t2 = tmp_pool.tile([HD, 1024], F32, name="t2", tag="tmp")
            nc.vector.tensor_mul(t1[:], dst[:, sl], cE[:, sl])
            nc.vector.tensor_mul(t2[:], sw[:], sE[:, sl])
            nc.vector.tensor_add(dst[:, sl], t1[:], t2[:])

    # constants needed next (emitted after Q's loads so Q starts sooner)
    nc.sync.dma_start(out=perm_t[:], in_=io["perm"][:])
    nc.sync.dma_start(out=cq_t[:], in_=io["cqE"][:])
    nc.sync.dma_start(out=sq_t[:], in_=io["sqE"][:])
    load_w_chunks(wk_all, io["wkT"])
    nc.sync.dma_start(out=bk_t[:], in_=io["bk4"][:])
    nc.sync.dma_start(out=ck_t[:], in_=io["ckE"][:])
    nc.sync.dma_start(out=sk_t[:], in_=io["skE"][:])

    # RoPE on q overlaps phase K below
    for h in range(NH_CORE):
        rope(qT[h], cq_t, sq_t, 0, TQ)

    # ---- phase K: k^T = Wk_h @ mem^T (c-outer, by Tk quarter) ------------
    with ExitStack() as kctx:
        psk_pool = kctx.enter_context(
            tc.tile_pool(name="psk", bufs=4, space="PSUM"))
        mh_pool = kctx.enter_context(tc.tile_pool(name="mhk", bufs=4))
        for q4 in range(4):
            col = q4 * 512
            psk = [psk_pool.tile([HD, 512], F32, name=f"psk{q4}_{h}", tag="psk")
                   for h in range(NH_CORE)]
            for c in range(KC):
                mc = mh_pool.tile([P, 512], QK_DT, name="mck", tag="mck")
                nc.sync.dma_start(
                    out=mc[:], in_=io["memT"][c * P:(c + 1) * P, col:col + 512])
                for h in range(NH_CORE):
                    nc.tensor.matmul(
                        psk[h][:], wk_all[:, c, h * HD:(h + 1) * HD], mc[:],
                        start=(c == 0), stop=(c == KC - 1))
            for h in range(NH_CORE):
                nc.scalar.activation(kT[h][:, col:col + 512],
                                     psk[h][:], Ident, bias=bk_t[:, h:h + 1])
            if q4 == 1:
                load_w_chunks(wv_all, io["wvT"])
            if q4 % 2 == 1:   # a full 1024-wide half is done -> rotate it
                for h in range(NH_CORE):
                    rope(kT[h], ck_t, sk_t, (q4 // 2) * 1024, 1024)

    sw_stack.close()

    # ---- phase V: v natural [Tk, 4*97] (c-outer, by Tk quarter) ----------
    with ExitStack() as vctx:
        psv_pool = vctx.enter_context(
            tc.tile_pool(name="psv", bufs=4, space="PSUM"))
        mv_pool = vctx.enter_context(tc.tile_pool(name="mhv", bufs=4))
        mem_v = io["memTv"] if SPLIT_MEM else io["memT"]
        for q4 in range(4):
            col = q4 * 512
            psv = [psv_pool.tile([P, HG], F32, name=f"psv{q4}_{i}", tag="psv")
                   for i in range(4)]
            for c in range(KC):
                mc = mv_pool.tile([P, 512], PV_DT, name="mcv", tag="mcv")
                nc.sync.dma_start(
                    out=mc[:], in_=mem_v[c * P:(c + 1) * P, col:col + 512])
                for ml in range(4):
                    nc.tensor.matmul(
                        psv[ml][:], mc[:, ml * P:(ml + 1) * P], wv_all[:, c, :],
                        start=(c == 0), stop=(c == KC - 1))
            for ml in range(4):
                mg = q4 * 4 + ml
                dst = vst[mg].rearrange("p (h c) -> p h c", c=VW)[:, :, 0:HD]
                src = psv[ml].rearrange("p (h c) -> p h c", c=HD)
                nc.vector.tensor_copy(dst, src)

    # ones columns of v + normalize/output constants
    for m in range(NKC):
        ones_cols = vst[m].rearrange("p (h c) -> p h c", c=VW)[:, :, HD:HD + 1]
        nc.sync.dma_start(out=ones_cols,
                          in_=io["ones4"][:].rearrange("p (h c) -> p h c", c=1))
    nc.sync.dma_start(out=ones1_t[:], in_=io["ones1"][:])
    wo_t = []
    for h in range(NH_CORE):
        w = const.tile([HD, D], PV_DT, name=f"wo_t{h}", tag=f"wo_t{h}")
        nc.sync.dma_start(out=w[:], in_=io["woT"][h * HD:(h + 1) * HD, :])
        wo_t.append(w)

    # ---- attention (software-pipelined: PV lags S/exp by 2 chunks) -------
    s_ps = ctx.enter_context(tc.tile_pool(name="s_ps", bufs=3, space="PSUM"))
    pv_ps = ctx.enter_context(tc.tile_pool(name="pv_ps", bufs=2, space="PSUM"))
    aout_pool = ctx.enter_context(tc.tile_pool(name="aout_pool", bufs=2))
    aN_pool = ctx.enter_context(tc.tile_pool(name="aN_pool", bufs=1))
    ot_pool = ctx.enter_context(tc.tile_pool(name="ot_pool", bufs=3))

    aoutN = [aN_pool.tile([HD, TQ], PV_DT, name=f"aoutN{h}", tag=f"aoutN{h}")
             for h in range(NH_CORE)]

    for h in range(NH_CORE):
        pv0 = pv_ps.tile([VW, 512], F32, name=f"pv{h}0", tag="pv")
        pv1 = pv_ps.tile([VW, 512], F32, name=f"pv{h}1", tag="pv")
        pts = [None] * NKC

        def emit_pv(k2):
            vl = vst[k2][:, h * VW:(h + 1) * VW]
            first, last = (k2 == 0), (k2 == NKC - 1)
            nc.tensor.matmul(pv0[:], vl, pts[k2][:, 0:512],
                             start=first, stop=last)
            nc.tensor.matmul(pv1[:], vl, pts[k2][:, 512:1024],
                             start=first, stop=last)

        for kc in range(NKC):
            st = s_ps.tile([P, TQ], F32, name="st", tag="s")
            lhs = kT[h][:, kc * P:(kc + 1) * P]
            nc.tensor.matmul(st[:, 0:512], lhs, qT[h][:, 0:512])
            nc.tensor.matmul(st[:, 512:1024], lhs, qT[h][:, 512:1024])
            pt = p_pool.tile([P, TQ], PV_DT, name="pt", tag="pt")
            nc.scalar.activation(pt[:], st[:], Exp, scale=SCALE)
            pts[kc] = pt
            if kc >= 2:
                emit_pv(kc - 2)
        emit_pv(NKC - 2)
        emit_pv(NKC - 1)

        aout = aout_pool.tile([VW, TQ], PV_DT, name="aout", tag="aout")
        nc.vector.tensor_copy(aout[:, 0:512], pv0[:])
        nc.vector.tensor_copy(aout[:, 512:1024], pv1[:])
        # broadcast the denominator row across partitions (K=1 matmul),
        # then full-lane approximate reciprocal and normalize
        den1 = tmp_pool.tile([1, TQ], PV_DT, name="den1", tag="den1")
        nc.vector.tensor_copy(den1[:], aout[HD:HD + 1, :])
        denB = s_ps.tile([P, TQ], F32, name="denB", tag="s")
        for n in range(2):
            nc.tensor.matmul(denB[:, n * 512:(n + 1) * 512], ones1_t[:],
                             den1[:, n * 512:(n + 1) * 512])
        recB = tmp_pool.tile([HD, TQ], F32, name="recB", tag="tmp")
        nc.vector.reciprocal_approx_fast(out=recB[:], in_=denB[0:HD, :])
        nc.vector.tensor_mul(aoutN[h][:], aout[0:HD, :], recB[:])

    # ---- output projection (h-inner accumulation, 2 matmuls per weight) --
    for m in range(MTILES):
        po0 = s_ps.tile([P, 512], F32, name="po0", tag="s")
        po1 = s_ps.tile([P, 512], F32, name="po1", tag="s")
        for h in range(NH_CORE):
            lhs = wo_t[h][:, m * P:(m + 1) * P]
            nc.tensor.matmul(po0[:], lhs, aoutN[h][:, 0:512],
                             start=(h == 0), stop=(h == NH_CORE - 1))
            nc.tensor.matmul(po1[:], lhs, aoutN[h][:, 512:1024],
                             start=(h == 0), stop=(h == NH_CORE - 1))
        ot = ot_pool.tile([P, TQ], F32, name="ot", tag="ot")
        nc.vector.tensor_copy(ot[:, 0:512], po0[:])
        nc.scalar.copy(ot[:, 512:1024], po1[:])
        nc.sync.dma_start(out=oT[m * P:(m + 1) * P, :], in_=ot[:])


# ---------------------------------------------------------------- host side
def _rope_tables(coords, T):
    """Feature-major cos/sin tables [HD, T] with the sign fold.

    Row j < 48 of the rotated output is q[j]*cos_j - q[j+48]*sin_j and row
    j >= 48 is q[j]*cos_{j-48} + q[j-48]*sin_{j-48}; the device computes
    rot = q * cE + swap(q) * sE with swap(q)[j] = q[(j+48) % 96].
    """
    coords = np.asarray(coords, np.float32)
    inv_freq = (1.0 / (ROPE_BASE ** (np.arange(FREQ_PER_AXIS, dtype=np.float32)
                                     / FREQ_PER_AXIS))).astype(np.float32)
    ang = coords[:, :, None] * inv_freq[None, None, :]   # [T, 3, 16]
    ang = ang.reshape(T, ROPE_HALF)                      # [T, 48]
    sin = np.sin(ang).astype(np.float32).T               # [48, T]
    cos = np.cos(ang).astype(np.float32).T
    cE = np.concatenate([cos, cos], axis=0)              # [96, T]
    sE = np.concatenate([-sin, sin], axis=0)
    return np.ascontiguousarray(cE), np.ascontiguousarray(sE)


def _make_in_maps(inputs):
    x = np.asarray(inputs["x"], np.float32)
    memory = np.asarray(inputs["memory"], np.float32)
    qc = np.asarray(inputs["query_coords"], np.float32)
    mc = np.asarray(inputs["memory_coords"], np.float32)
    Wq = np.asarray(inputs["Wq"], np.float32)
    Wk = np.asarray(inputs["Wk"], np.float32)
    Wv = np.asarray(inputs["Wv"], np.float32)
    Wo = np.asarray(inputs["Wo"], np.float32)
    bq = np.asarray(inputs["bq"], np.float32)
    bk = np.asarray(inputs["bk"], np.float32)

    WqT = np.ascontiguousarray(Wq.T).astype(QK_NP)   # [in, out]
    WkT = np.ascontiguousarray(Wk.T).astype(QK_NP)
    WvT = np.ascontiguousarray(Wv.T).astype(PV_NP)
    WoT = np.ascontiguousarray(Wo.T).astype(PV_NP)

    perm = np.zeros((HD, HD), QK_NP)   # lhsT: perm[j, i] = [j == (i+48)%96]
    for i in range(HD):
        perm[(i + ROPE_HALF) % HD, i] = 1.0

    per_batch = []
    for b in range(B):
        cqE, sqE = _rope_tables(qc[b], TQ)
        ckE, skE = _rope_tables(mc[b], TK)
        entry = {
            "xT": np.ascontiguousarray(x[b].T).astype(QK_NP),
            "memT": np.ascontiguousarray(memory[b].T).astype(QK_NP),
            "cqE": cqE, "sqE": sqE, "ckE": ckE, "skE": skE,
        }
        if SPLIT_MEM:
            entry["memTv"] = np.ascontiguousarray(memory[b].T).astype(PV_NP)
        per_batch.append(entry)

    in_maps = []
    for core in range(N_CORES):
        b, g = divmod(core, NH_CORE)
        sl = slice(g * HG, (g + 1) * HG)
        m = dict(per_batch[b])
        m["wqT"] = np.ascontiguousarray(WqT[:, sl])
        m["wkT"] = np.ascontiguousarray(WkT[:, sl])
        m["wvT"] = np.ascontiguousarray(WvT[:, sl])
        m["woT"] = np.ascontiguousarray(WoT[sl, :])
        m["bq4"] = np.ascontiguousarray(bq[sl].reshape(NH_CORE, HD).T)
        m["bk4"] = np.ascontiguousarray(bk[sl].reshape(NH_CORE, HD).T)
        m["perm"] = perm
        m["ones1"] = np.ones((1, 128), PV_NP)
        m["ones4"] = np.ones((128, NH_CORE), PV_NP)
        in_maps.append(m)
    return in_maps


def _assemble(results, inputs):
    Wo = np.asarray(inputs["Wo"], np.float32)
    bv = np.asarray(inputs["bv"], np.float32)
    bo = np.asarray(inputs["bo"], np.float32)
    cvec = (bv @ Wo.T + bo).astype(np.float32)   # exact: attn rows sum to 1
    out = np.empty((B, TQ, D), np.float32)
    for b in range(B):
        acc = np.zeros((D, TQ), np.float64)
        for g in range(NH_CORE):
            acc += results[b * NH_CORE + g]["oT"]
        out[b] = acc.T.astype(np.float32) + cvec
    return out


_NC_CACHE = None


def _get_nc():
    global _NC_CACHE
    if _NC_CACHE is None:
        _NC_CACHE = _build_nc()
    return _NC_CACHE


def kernel(**inputs) -> np.ndarray:
    nc = _get_nc()
    in_maps = _make_in_maps(inputs)
    res = run_bass_kernel_spmd(nc, in_maps, list(range(N_CORES)))
    return _assemble(res.results, inputs)
